# revision 1
# baseline (speedup 1.0000x reference)
"""Trainium2 Bass kernel for nn_Causal_Attention_13082470383895.

Full (unsharded) inputs in, full output out. Internally shards batch*heads
across 8 NeuronCores: core c owns batch c//4 and the 4 heads [4*(c%4), 4*(c%4)+4).
Each core computes its heads' q/k/v projections (column-sharded weights),
QK-layernorm, causal unnormalized-exp attention, and its partial contribution
to the output projection (row-sharded W_out). Host sums the 4 partials per batch.

Hardcoded shapes (per spec): inputs [2, 2048, 1024], W_qk [1024, 2048],
W_v [1024, 1024], W_out [1024, 1024], q/k scale=ones, bias=zeros (per spec
fill; layernorm affine is identity and is not applied).
"""

import os
import sys

import numpy as np

sys.path.insert(0, "/opt/trn_rl_repo")

B = 2
L = 2048
D = 1024
HEADS = 16
DIM = 64
LN_EPS = 1e-6
P = 128
LT = L // P          # 16 l-tiles
DT = D // P          # 8 contraction tiles
NHL = 4              # heads per core
SUP = 4              # 512-wide l_q supertiles
N_CORES = 8

_CACHE = {}


def _make_bacc_cls():
    import bass_rust
    import concourse.mybir as mybir
    from concourse import bacc
    from concourse.hw_specs import get_activation_tables

    class KernelBacc(bacc.Bacc):
        """Bacc whose ACT-table selector never picks the `natural_log` set
        for Ln: hiding `ln` there makes the greedy selector choose
        `natural_log_exp_and_others` (which also holds exp/copy), so the
        kernel needs a single table load instead of thrashing
        exp_and_others <-> natural_log on every layernorm."""

        def insert_act_table_loads(self):
            has_activation = any(
                isinstance(i, mybir.InstActivation)
                for b in self.main_func.blocks
                for i in b.instructions
            )
            if not has_activation:
                return
            ln = mybir.ActivationFunctionType.Ln
            tables = []
            for name, funcs in get_activation_tables(self.m.arch).items():
                if name == "natural_log":
                    funcs = funcs - {ln}
                tables.append((name, funcs))
            bass_rust.insert_act_table_loads(self, tables)

    return KernelBacc


def _build_nc():
    import concourse.bass as bass
    import concourse.mybir as mybir
    import concourse.tile as tile
    from concourse.masks import (
        make_identity,
        make_lower_triangular,
        make_upper_triangular,
    )

    f32 = mybir.dt.float32
    f32r = mybir.dt.float32r
    AF = mybir.ActivationFunctionType
    ALU = mybir.AluOpType

    nc = _make_bacc_cls()("TRN2", target_bir_lowering=False, debug=False)

    X = nc.dram_tensor("x", [L, D], f32, kind="ExternalInput").ap()
    WQK = nc.dram_tensor("w_qk", [D, 512], f32, kind="ExternalInput").ap()
    WV = nc.dram_tensor("w_v", [D, 256], f32, kind="ExternalInput").ap()
    WOUT = nc.dram_tensor("w_out", [256, D], f32, kind="ExternalInput").ap()
    OUT = nc.dram_tensor("out", [L, D], f32, kind="ExternalOutput").ap()

    with tile.TileContext(nc) as tc:
        const = tc.alloc_tile_pool(name="const", bufs=1)
        big = tc.alloc_tile_pool(name="big", bufs=1)
        work = tc.alloc_tile_pool(name="work", bufs=2)
        stat = tc.alloc_tile_pool(name="stat", bufs=3)
        esp = tc.alloc_tile_pool(name="esp", bufs=4)
        outp = tc.alloc_tile_pool(name="outp", bufs=2)

        ident = const.tile([P, P], f32)
        make_identity(nc, ident)
        # S^T layout: element (lk, lq) valid iff lq >= lk. Additive mask
        # applied to scores BEFORE exp: 0 where valid, -1e30 below diagonal.
        maskn = const.tile([P, P], f32)
        make_lower_triangular(nc, maskn, val=-1e30, diag=False)
        ones_f32 = const.tile([P, 1], f32)
        nc.vector.memset(ones_f32, 1.0)
        ones_row = const.tile([1, DIM], f32r)
        nc.vector.tensor_copy(ones_row, ones_f32[0:1, :].to_broadcast([1, DIM]))
        epsb = const.tile([P, 1], f32)
        nc.vector.memset(epsb, float(D * LN_EPS))
        # keep the upper-triangular 0/1 mask for post-exp causal masking
        up01 = const.tile([P, P], f32)
        make_upper_triangular(nc, up01, val=1.0, diag=True)

        # Collapse const-setup waits behind one barrier (wait-slot limits).
        tc.strict_bb_all_engine_barrier()

        # First x tiles before the (bigger) weight DMAs so PE can start
        # transposing immediately.
        x_tiles = {}
        for t in range(2):
            x_t = work.tile([P, D], f32, tag="x", bufs=3, name="x_t")
            nc.sync.dma_start(x_t, X[t * P:(t + 1) * P, :])
            x_tiles[t] = x_t

        # weights: DMA f32, then cast to fp32r (PE operands must be produced
        # as rounded fp32r). wqk cast is chunked so the first projection can
        # start as soon as chunk 0 is ready.
        wqk_f = work.tile([P, DT, 512], f32, tag="wstage", bufs=1)
        nc.sync.dma_start(wqk_f, WQK.rearrange("(o p) n -> p o n", p=P))
        wqk = big.tile([P, DT, 512], f32r)
        for d in range(DT):
            nc.vector.tensor_copy(wqk[:, d], wqk_f[:, d])
        wv_f = work.tile([P, DT, 256], f32, tag="wstage", bufs=1)
        nc.sync.dma_start(wv_f, WV.rearrange("(o p) n -> p o n", p=P))
        wv = big.tile([P, DT, 256], f32r)
        nc.vector.tensor_copy(wv, wv_f)
        wout_f = work.tile([P, 2, D], f32, tag="wstage", bufs=1)
        nc.sync.dma_start(wout_f, WOUT.rearrange("(c p) n -> p c n", p=P))
        wout = big.tile([P, 2, D], f32r)
        nc.vector.tensor_copy(wout, wout_f)

        # persistent intermediates. qt/kt/at pair 2 heads on the partition
        # axis: head 2i in rows 0:64, head 2i+1 in rows 64:128.
        # v is stored augmented per head: [v_h | 1] (65 cols) so one AV
        # matmul yields both the numerator (rows 0:64) and the softmax
        # denominator (row 64).
        v_sb = big.tile([P, LT, NHL, DIM + 1], f32r)
        qt = [big.tile([P, L], f32r, name=f"qt{i}") for i in range(2)]
        kt = [big.tile([P, L], f32r, name=f"kt{i}") for i in range(2)]
        at = [big.tile([P, L], f32r, name=f"at{i}") for i in range(2)]
        # ones column of every v_aug tile (produced as rounded f32r via DVE)
        nc.vector.tensor_copy(
            v_sb[:, :, :, DIM],
            ones_f32[:, 0:1].to_broadcast([P, LT, NHL]),
        )

        # One shared PSUM pool: tags sized so phases A and B can overlap.
        # b512 slots serve xt/proj/qkt/st/av/bc tiles; op gets its own 2
        # banks. 6 + 2 = 8 banks.
        with tc.tile_pool(name="ps", bufs=6, space="PSUM") as ps:
            # Per-supertile: phase A (projections+LN+transposes) for s, then
            # phase B (attention) and C (out-projection) for s — interleaved
            # in program order so the per-engine FIFO streams overlap.
            def phase_a(s):
                qk_tiles = []
                for t in range(4 * s, 4 * s + 4):
                    if t in x_tiles:
                        x_t = x_tiles.pop(t)
                    else:
                        x_t = work.tile([P, D], f32, tag="x", bufs=3,
                                        name="x_t")
                        nc.sync.dma_start(x_t, X[t * P:(t + 1) * P, :])

                    # transpose x tile -> x^T chunks [d, l]
                    xt_sb = work.tile([P, DT, P], f32r, tag="xt_sb")
                    for half in range(2):
                        xt_ps = ps.tile([P, 512], f32, tag="b512",
                                        name="xt_ps")
                        for dj in range(4):
                            d = half * 4 + dj
                            nc.tensor.transpose(
                                xt_ps[:, dj * P:(dj + 1) * P],
                                x_t[:, d * P:(d + 1) * P],
                                ident,
                            )
                        if half == 0:
                            nc.scalar.copy(
                                xt_sb[:, :4, :],
                                xt_ps.rearrange("p (a b) -> p a b", a=4),
                            )
                        else:
                            nc.vector.tensor_copy(
                                xt_sb[:, 4:, :],
                                xt_ps.rearrange("p (a b) -> p a b", a=4),
                            )

                    # qk / v projections (contract over D)
                    qk_ps = ps.tile([P, 512], f32, tag="b512", name="qk_ps")
                    v_ps = ps.tile([P, 512], f32, tag="b512", name="v_ps")
                    for d in range(DT):
                        nc.tensor.matmul(
                            qk_ps, xt_sb[:, d], wqk[:, d],
                            start=(d == 0), stop=(d == DT - 1),
                        )
                    for d in range(DT):
                        nc.tensor.matmul(
                            v_ps[:, :256], xt_sb[:, d], wv[:, d],
                            start=(d == 0), stop=(d == DT - 1),
                        )
                    # 72-wide groups: pad so per-group APs stay 3D
                    qk_full = work.tile([P, 8, DIM + 8], f32, tag="qk_sb",
                                        bufs=6)
                    qk_sb = qk_full[:, :, :DIM]
                    nc.vector.tensor_copy(
                        qk_sb, qk_ps.rearrange("p (g d) -> p g d", g=8))
                    nc.vector.tensor_copy(
                        v_sb[:, t, :, :DIM],
                        v_ps[:, :256].rearrange("p (h d) -> p h d", h=NHL))

                    # layernorm over each 64-group. qk is RAW (unscaled by
                    # 1/32): (raw-m)/sqrt(var_raw + 1024*eps) matches the
                    # reference exactly.
                    bnst_full = stat.tile([P, 8, 8], f32, tag="bnst")
                    bnst = bnst_full[:, :, :6]
                    mv = stat.tile([P, 8, 2], f32, tag="mv")
                    for g in range(8):
                        nc.vector.bn_stats(bnst[:, g], qk_sb[:, g])
                        nc.vector.bn_aggr(mv[:, g], bnst[:, g])
                    rstd = stat.tile([P, 8], f32, tag="rstd")
                    nc.scalar.activation(rstd, mv[:, :, 1], AF.Ln,
                                         bias=epsb, scale=1.0)
                    nc.scalar.activation(rstd, rstd, AF.Exp, scale=-0.5)
                    prod = stat.tile([P, 8], f32, tag="prod")
                    nc.vector.tensor_tensor(prod, mv[:, :, 0], rstd, ALU.mult)
                    for g in range(8):
                        nc.gpsimd.tensor_scalar(
                            qk_sb[:, g], qk_sb[:, g],
                            rstd[:, g:g + 1], prod[:, g:g + 1],
                            op0=ALU.mult, op1=ALU.subtract,
                        )
                    qk_tiles.append(qk_sb)

                # transpose q_n, k_n -> [dim, l] for this supertile's 4
                # l-tiles. Matmul outputs must start at PSUM partition 0, so
                # transpose into [64, 512] tiles and pair heads during the
                # SBUF copy.
                for hl in range(NHL):
                    pr, ro = hl // 2, DIM * (hl % 2)
                    for which, dst in ((0, qt), (1, kt)):
                        tp_ps = ps.tile([DIM, 512], f32, tag="b512",
                                        name="tp_ps")
                        for i in range(4):
                            nc.tensor.transpose(
                                tp_ps[:, i * P:(i + 1) * P],
                                qk_tiles[i][:, 2 * hl + which],
                                ident,
                            )
                        nc.vector.tensor_copy(
                            dst[pr][ro:ro + DIM, s * 512:(s + 1) * 512],
                            tp_ps,
                        )

            def phase_bc(s):
                ls = slice(s * 512, (s + 1) * 512)
                njs = 4 * s + 4
                for pr in range(2):
                    # two heads interleaved: disjoint PE row groups (0:64 /
                    # 64:128) let their K=64 QK matmuls run concurrently
                    av_list = []
                    for r01 in range(2):
                        av_list.append(ps.tile([DIM + 1, 512], f32,
                                               tag="b512",
                                               name=f"av_ps{r01}"))
                    for j in range(njs):
                        pp = j - 4 * s  # >=0: diagonal tile needing mask
                        woff = max(0, pp) * P
                        es_list = []
                        for r01 in range(2):
                            ro = DIM * r01
                            st_ps = ps.tile([P, 512], f32, tag="b512",
                                            name=f"st_ps{r01}")
                            nc.tensor.matmul(
                                st_ps,
                                kt[pr][ro:ro + DIM, j * P:(j + 1) * P],
                                qt[pr][ro:ro + DIM, ls],
                                start=True, stop=True, tile_position=(ro, 0),
                            )
                            es = esp.tile([P, 512], f32r, tag="es")
                            nc.scalar.activation(es[:, woff:],
                                                 st_ps[:, woff:],
                                                 AF.Exp, scale=1.0 / DIM)
                            if pp >= 0:
                                blk = slice(pp * P, (pp + 1) * P)
                                nc.gpsimd.tensor_tensor(
                                    es[:, blk], es[:, blk], up01, ALU.mult)
                            es_list.append(es)
                        for r01 in range(2):
                            hl = 2 * pr + r01
                            nc.tensor.matmul(
                                av_list[r01][:, woff:],
                                v_sb[:, j, hl],
                                es_list[r01][:, woff:],
                                start=(j == 0), stop=(j == njs - 1),
                            )
                    for r01 in range(2):
                        hl = 2 * pr + r01
                        ro = DIM * r01
                        av_ps = av_list[r01]
                        recip = stat.tile([1, 512], f32r, tag="recip")
                        with nc.allow_low_precision(
                                reason="fp32r rounding of softmax recip"):
                            nc.vector.reciprocal(recip, av_ps[DIM:DIM + 1, :])
                        bc_ps = ps.tile([DIM, 512], f32, tag="b512",
                                        name="bc_ps")
                        nc.tensor.matmul(bc_ps, ones_row, recip,
                                         start=True, stop=True)
                        # DVE reads at most one PSUM operand: stage av via
                        # ScalarE
                        av_sb = esp.tile([DIM, 512], f32, tag="avsb")
                        nc.scalar.copy(av_sb, av_ps[:DIM])
                        nc.vector.tensor_tensor(at[pr][ro:ro + DIM, ls],
                                                av_sb, bc_ps, ALU.mult)

            def phase_c(s):
                # output projection for supertile s's l-tiles
                for t in range(4 * s, 4 * s + 4):
                    op_ps = ps.tile([P, D], f32, tag="op", bufs=1,
                                    name="op_ps")
                    for nch in range(2):
                        for c in range(2):
                            nc.tensor.matmul(
                                op_ps[:, nch * 512:(nch + 1) * 512],
                                at[c][:, t * P:(t + 1) * P],
                                wout[:, c, nch * 512:(nch + 1) * 512],
                                start=(c == 0), stop=(c == 1),
                            )
                    o_sb = outp.tile([P, D], f32, tag="o")
                    # 1/32 (v proj) * 1/32 (out proj) = 1/1024
                    nc.scalar.mul(o_sb, op_ps, 1.0 / 1024.0)
                    nc.sync.dma_start(OUT[t * P:(t + 1) * P, :], o_sb)

            for s in range(SUP):
                phase_a(s)
                phase_bc(s)
                if s > 0:
                    phase_c(s - 1)
            phase_c(SUP - 1)

        outp.release()
        esp.release()
        stat.release()
        work.release()
        big.release()
        const.release()

    nc.finalize()
    return nc


def _get_nc():
    if "nc" not in _CACHE:
        _CACHE["nc"] = _build_nc()
    return _CACHE["nc"]


def kernel(**inputs):
    x = np.ascontiguousarray(np.asarray(inputs["inputs"], dtype=np.float32))
    w_qk = np.asarray(inputs["W_qk"], dtype=np.float32)
    w_v = np.asarray(inputs["W_v"], dtype=np.float32)
    w_out = np.asarray(inputs["W_out"], dtype=np.float32)

    nc = _get_nc()
    in_maps = []
    for c in range(N_CORES):
        b, g = divmod(c, 4)
        in_maps.append({
            "x": np.ascontiguousarray(x[b]),
            "w_qk": np.ascontiguousarray(w_qk[:, 512 * g:512 * (g + 1)]),
            "w_v": np.ascontiguousarray(w_v[:, 256 * g:256 * (g + 1)]),
            "w_out": np.ascontiguousarray(w_out[256 * g:256 * (g + 1), :]),
        })

    from concourse.bass_utils import run_bass_kernel_spmd

    trace = bool(os.environ.get("KERNEL_TRACE"))
    if trace:
        try:
            from antenv.axon_hooks import get_axon_ntff_profile_hook  # noqa: F401
        except Exception:
            trace = False
    res = run_bass_kernel_spmd(nc, in_maps, core_ids=list(range(N_CORES)),
                               trace=trace)
    _CACHE["last_results"] = res
    outs = [m["out"] for m in res.results]
    out = np.stack([
        outs[0] + outs[1] + outs[2] + outs[3],
        outs[4] + outs[5] + outs[6] + outs[7],
    ]).astype(np.float32)
    return out



# revision 15
# speedup vs baseline: 1.9033x; 1.9033x over previous
"""Trainium2 Bass kernel for nn_Causal_Attention_13082470383895.

Full (unsharded) inputs in, full output out. Internally shards batch*heads
across 8 NeuronCores: core c owns batch c//4 and the 4 heads [4*(c%4), 4*(c%4)+4).
Each core computes its heads' q/k/v projections, QK-layernorm, causal
unnormalized-exp attention, and its partial contribution to the output
projection (row-sharded W_out). Host sums the 4 partials per batch.

All matmul operands are bf16 (PE runs 1 cycle/row vs 2-4 for fp32 modes).
Host-side prep: x is pre-transposed to [D, L] (removes 128 on-chip PE
transposes), weights are pre-scaled by 1/32 so no device-side scaling is
needed, and W_qk columns are permuted to [q0,q1,k0,k1,q2,q3,k2,k3] so each
128-wide transpose slab directly yields a head-paired q^T/k^T tile.

Hardcoded shapes (per spec): inputs [2, 2048, 1024], W_qk [1024, 2048],
W_v [1024, 1024], W_out [1024, 1024], q/k scale=ones, bias=zeros (per spec
fill; layernorm affine is identity and is not applied).
"""

import os
import sys

import numpy as np

sys.path.insert(0, "/opt/trn_rl_repo")

B = 2
L = 2048
D = 1024
HEADS = 16
DIM = 64
LN_EPS = 1e-6
P = 128
LT = L // P          # 16 l-tiles
DT = D // P          # 8 contraction tiles
NHL = 4              # heads per core
SUP = 4              # 512-wide l_q supertiles
N_CORES = 8

_CACHE = {}


def _make_bacc_cls():
    import bass_rust
    import concourse.mybir as mybir
    from concourse import bacc
    from concourse.hw_specs import get_activation_tables

    class KernelBacc(bacc.Bacc):
        """Bacc whose ACT-table selector never picks the `natural_log` set
        for Ln: hiding `ln` there makes the greedy selector choose
        `natural_log_exp_and_others` (which also holds exp/copy), so the
        kernel needs a single table load instead of thrashing
        exp_and_others <-> natural_log on every layernorm."""

        def insert_act_table_loads(self):
            has_activation = any(
                isinstance(i, mybir.InstActivation)
                for b in self.main_func.blocks
                for i in b.instructions
            )
            if not has_activation:
                return
            ln = mybir.ActivationFunctionType.Ln
            tables = []
            for name, funcs in get_activation_tables(self.m.arch).items():
                if name == "natural_log":
                    funcs = funcs - {ln}
                tables.append((name, funcs))
            bass_rust.insert_act_table_loads(self, tables)

    return KernelBacc


def _build_nc():
    import concourse.bass as bass
    import concourse.mybir as mybir
    import concourse.tile as tile
    from concourse.masks import make_identity, make_upper_triangular

    f32 = mybir.dt.float32
    f32r = mybir.dt.float32r
    bf16 = mybir.dt.bfloat16
    AF = mybir.ActivationFunctionType
    ALU = mybir.AluOpType

    nc = _make_bacc_cls()("TRN2", target_bir_lowering=False, debug=False)

    debug = bool(os.environ.get("KERNEL_DEBUG"))
    XT = nc.dram_tensor("xt", [D, L], bf16, kind="ExternalInput").ap()
    WQK = nc.dram_tensor("w_qk", [D, 512], bf16, kind="ExternalInput").ap()
    # w_v is augmented with 8 columns of per-group column-sums of w_qk, so
    # the v projection also yields the layernorm group sums for free.
    WV = nc.dram_tensor("w_v", [D, 264], bf16, kind="ExternalInput").ap()
    WOUT = nc.dram_tensor("w_out", [256, D], bf16, kind="ExternalInput").ap()
    OUT = nc.dram_tensor("out", [L, D], bf16, kind="ExternalOutput").ap()
    if debug:
        DBG_QT = nc.dram_tensor("dbg_qt", [P, L], bf16,
                                kind="ExternalOutput").ap()
        DBG_KT = nc.dram_tensor("dbg_kt", [P, L], bf16,
                                kind="ExternalOutput").ap()
        DBG_AT = nc.dram_tensor("dbg_at", [P, L], bf16,
                                kind="ExternalOutput").ap()
        DBG_V = nc.dram_tensor("dbg_v", [P, LT * NHL * (DIM + 1)], bf16,
                               kind="ExternalOutput").ap()

    with tile.TileContext(nc) as tc:
        const = tc.alloc_tile_pool(name="const", bufs=1)
        big = tc.alloc_tile_pool(name="big", bufs=1)
        qkp = tc.alloc_tile_pool(name="qkp", bufs=5)
        stat = tc.alloc_tile_pool(name="stat", bufs=3)
        esp = tc.alloc_tile_pool(name="esp", bufs=3)
        tailp = tc.alloc_tile_pool(name="tailp", bufs=2)
        outp = tc.alloc_tile_pool(name="outp", bufs=4)

        ident = const.tile([P, P], bf16)
        make_identity(nc, ident)
        # 0/1 upper-triangular (incl diag) mask for post-exp causal masking
        # of diagonal score tiles; layout (k_partition, q_free), valid q>=k.
        up01 = const.tile([P, P], bf16)
        make_upper_triangular(nc, up01, val=1.0, diag=True)
        ones_row = const.tile([1, DIM], f32)
        nc.vector.memset(ones_row, 1.0)
        epsb = const.tile([P, 1], f32)
        nc.vector.memset(epsb, float(LN_EPS))

        tc.strict_bb_all_engine_barrier()

        # --- weights + x^T DMA (bf16 straight from host, no casts) -------
        wqk = big.tile([P, DT, 512], bf16)
        xt_all = big.tile([P, DT, L], bf16)
        # interleave wqk chunks and first-half x chunks so the first
        # projection can start as soon as chunk 0 of each is resident
        for d in range(DT):
            nc.sync.dma_start(wqk[:, d], WQK[d * P:(d + 1) * P, :])
            nc.sync.dma_start(xt_all[:, d, :1024], XT[d * P:(d + 1) * P, :1024])
        wv = big.tile([P, DT, 264], bf16)
        for d in range(DT):
            nc.sync.dma_start(wv[:, d], WV[d * P:(d + 1) * P, :])
        wout = big.tile([P, 2, D], bf16)
        for c in range(2):
            nc.sync.dma_start(wout[:, c], WOUT[c * P:(c + 1) * P, :])
        for d in range(DT):
            nc.sync.dma_start(xt_all[:, d, 1024:], XT[d * P:(d + 1) * P, 1024:])

        # persistent intermediates. qt/kt/at pair 2 heads on the partition
        # axis: local head 2i in rows 0:64, head 2i+1 in rows 64:128.
        # v is stored augmented per head: [v_h | 1] (65 cols) so one AV
        # matmul yields both the numerator (rows 0:64) and the softmax
        # denominator (row 64).
        v_sb = big.tile([P, LT, NHL, DIM + 1], bf16)
        nc.vector.memset(v_sb[:, :, :, DIM], 1.0)
        qt = [big.tile([P, L], bf16, name=f"qt{i}") for i in range(2)]
        kt = [big.tile([P, L], bf16, name=f"kt{i}") for i in range(2)]
        at = [big.tile([P, L], bf16, name=f"at{i}") for i in range(2)]

        # PSUM: tag "big" [P,2,512]f32 = 2 banks x bufs2 = 4 (st tiles in
        # the attention j-loop, op tiles in the out-projection); tag "av"
        # [65,512]f32 1 bank x bufs2 = 2; tag "work" [P,512]f32 1 bank x
        # bufs2 = 2 (qk/v projections and q/k transposes). Total 8 banks.
        with tc.tile_pool(name="ps", bufs=2, space="PSUM") as ps:

            def phase_a(s):
                qk_tiles = []
                for t in range(4 * s, 4 * s + 4):
                    # v projection first: its last 8 cols are the layernorm
                    # group sums of the (not yet computed) qk projection
                    v_ps = ps.tile([P, 512], f32, tag="work", name="v_ps")
                    for d in range(DT):
                        nc.tensor.matmul(
                            v_ps[:, :264], xt_all[:, d, t * P:(t + 1) * P],
                            wv[:, d],
                            start=(d == 0), stop=(d == DT - 1),
                        )
                    nc.vector.tensor_copy(
                        v_sb[:, t, :, :DIM],
                        v_ps[:, :256].rearrange("p (h d) -> p h d", h=NHL))
                    sums = v_ps[:, 256:264]
                    mean = stat.tile([P, 8, 1], f32, tag="mean")
                    nc.vector.tensor_scalar(mean[:, :, 0], sums,
                                            1.0 / DIM, None, op0=ALU.mult)

                    qk_ps = ps.tile([P, 512], f32, tag="work", name="qk_ps")
                    for d in range(DT):
                        nc.tensor.matmul(
                            qk_ps, xt_all[:, d, t * P:(t + 1) * P],
                            wqk[:, d],
                            start=(d == 0), stop=(d == DT - 1),
                        )
                    # grouped layernorm: var = (sumsq - sums*mean)/64 + eps,
                    # the /64 folded into the Ln scale. Apply is two
                    # full-width ops with [P,8,1]->[P,8,64] broadcasts.
                    qk_g = qk_ps.rearrange("p (g d) -> p g d", g=8)
                    sq = qkp.tile([P, 512], f32, tag="sq", bufs=2, name="sq")
                    nc.scalar.activation(sq, qk_ps, AF.Square, scale=1.0)
                    sumsq = stat.tile([P, 8], f32, tag="sumsq")
                    nc.vector.tensor_reduce(
                        sumsq, sq.rearrange("p (g d) -> p g d", g=8),
                        mybir.AxisListType.X, ALU.add)
                    u = stat.tile([P, 8], f32, tag="u")
                    nc.vector.tensor_tensor(u, sums, mean[:, :, 0], ALU.mult)
                    nc.vector.tensor_tensor(u, sumsq, u, ALU.subtract)
                    rstd = stat.tile([P, 8, 1], f32, tag="rstd")
                    nc.scalar.activation(rstd[:, :, 0], u, AF.Ln,
                                         bias=epsb, scale=1.0 / DIM)
                    nc.scalar.activation(rstd[:, :, 0], rstd[:, :, 0],
                                         AF.Exp, scale=-0.5)
                    qk_sb = qkp.tile([P, 512], bf16, tag="qk", name="qk_sb")
                    qk_sbg = qk_sb.rearrange("p (g d) -> p g d", g=8)
                    cen = qkp.tile([P, 512], f32, tag="cen", bufs=2,
                                   name="cen")
                    ceng = cen.rearrange("p (g d) -> p g d", g=8)
                    nc.vector.tensor_tensor(
                        ceng, qk_g, mean.to_broadcast([P, 8, DIM]),
                        ALU.subtract)
                    nc.vector.tensor_tensor(
                        qk_sbg, ceng, rstd.to_broadcast([P, 8, DIM]),
                        ALU.mult)
                    qk_tiles.append(qk_sb)

                # q/k transposes. Host permuted W_qk columns so slab a of
                # qk_sb is [q_even|q_odd] or [k_even|k_odd] of a head pair:
                # one [128,128] transpose yields the paired-layout tile.
                for a, dst in ((0, qt[0]), (1, kt[0]), (2, qt[1]), (3, kt[1])):
                    tp = ps.tile([P, 512], bf16, tag="work", name="tp_ps")
                    for i in range(4):
                        nc.tensor.transpose(
                            tp[:, i * P:(i + 1) * P],
                            qk_tiles[i][:, a * P:(a + 1) * P],
                            ident,
                        )
                    nc.vector.tensor_copy(dst[:, s * 512:(s + 1) * 512], tp)

            def phase_bc(s):
                njs = 4 * s + 4
                for pr in range(2):
                    av = [ps.tile([DIM + 1, 512], f32, tag="av",
                                  name=f"av{r01}") for r01 in range(2)]
                    for j in range(njs):
                        pp = j - 4 * s
                        woff = max(0, pp) * P
                        st = ps.tile([P, 2, 512], f32, tag="big", name="st")
                        for r01 in range(2):
                            ro = DIM * r01
                            nc.tensor.matmul(
                                st[:, r01, woff:],
                                kt[pr][ro:ro + DIM, j * P:(j + 1) * P],
                                qt[pr][ro:ro + DIM,
                                       s * 512 + woff:(s + 1) * 512],
                                start=True, stop=True, tile_position=(ro, 0),
                            )
                        es = esp.tile([P, 2, 512], bf16, tag="es")
                        nc.scalar.activation(es[:, :, woff:], st[:, :, woff:],
                                             AF.Exp, scale=1.0 / DIM)
                        if pp >= 0:
                            blk = slice(pp * P, (pp + 1) * P)
                            nc.gpsimd.tensor_tensor(
                                es[:, :, blk], es[:, :, blk],
                                up01.unsqueeze(1).to_broadcast([P, 2, P]),
                                ALU.mult)
                        for r01 in range(2):
                            nc.tensor.matmul(
                                av[r01][:, woff:],
                                v_sb[:, j, 2 * pr + r01],
                                es[:, r01, woff:],
                                start=(j == 0), stop=(j == njs - 1),
                            )
                    for r01 in range(2):
                        ro = DIM * r01
                        # custom-DVE recip mis-reads PSUM at a nonzero base
                        # partition on HW (sim disagrees): stage the denom
                        # row to a base-0 SBUF tile first.
                        den = stat.tile([1, 512], f32, tag="den")
                        nc.vector.tensor_copy(den, av[r01][DIM:DIM + 1, :])
                        rec = stat.tile([1, 512], f32, tag="rec")
                        nc.vector.reciprocal_approx_fast(rec, den)
                        bc = ps.tile([DIM, 512], f32, tag="big", name="bc")
                        nc.tensor.matmul(bc, ones_row, rec,
                                         start=True, stop=True)
                        av_sb = tailp.tile([DIM, 512], f32, tag="avsb")
                        nc.vector.tensor_copy(av_sb, av[r01][:DIM])
                        nc.vector.tensor_tensor(
                            at[pr][ro:ro + DIM, s * 512:(s + 1) * 512],
                            av_sb, bc, ALU.mult)

            def phase_c(s):
                for t in range(4 * s, 4 * s + 4):
                    op = ps.tile([P, 2, 512], f32, tag="big", name="op")
                    for nch in range(2):
                        for c in range(2):
                            nc.tensor.matmul(
                                op[:, nch],
                                at[c][:, t * P:(t + 1) * P],
                                wout[:, c, nch * 512:(nch + 1) * 512],
                                start=(c == 0), stop=(c == 1),
                            )
                    o_sb = outp.tile([P, D], bf16, tag="o")
                    nc.scalar.copy(o_sb, op.rearrange("p a b -> p (a b)"))
                    nc.sync.dma_start(OUT[t * P:(t + 1) * P, :], o_sb)

            for s in range(SUP):
                phase_a(s)
                phase_bc(s)
                if s > 0:
                    phase_c(s - 1)
            phase_c(SUP - 1)
            if debug:
                nc.sync.dma_start(DBG_QT, qt[0])
                nc.sync.dma_start(DBG_KT, kt[0])
                nc.sync.dma_start(DBG_AT, at[0])
                nc.sync.dma_start(
                    DBG_V, v_sb.rearrange("p a b c -> p (a b c)"))

        outp.release()
        tailp.release()
        esp.release()
        stat.release()
        qkp.release()
        big.release()
        const.release()

    nc.finalize()
    return nc


def _get_nc():
    if "nc" not in _CACHE:
        _CACHE["nc"] = _build_nc()
    return _CACHE["nc"]


def kernel(**inputs):
    import ml_dtypes

    bf = ml_dtypes.bfloat16
    x = np.asarray(inputs["inputs"], dtype=np.float32)
    w_qk = np.asarray(inputs["W_qk"], dtype=np.float32) * (1.0 / 32.0)
    w_v = (np.asarray(inputs["W_v"], dtype=np.float32) * (1.0 / 32.0)).astype(bf)
    w_out = (np.asarray(inputs["W_out"], dtype=np.float32) * (1.0 / 32.0)).astype(bf)

    # permute each core's 8 qk column-groups [q0,k0,q1,k1,q2,k2,q3,k3] ->
    # [q0,q1,k0,k1,q2,q3,k2,k3] so a 128-col transpose slab is a head pair
    perm = [0, 2, 1, 3, 4, 6, 5, 7]
    xT = [np.ascontiguousarray(x[b].T).astype(bf) for b in range(B)]

    nc = _get_nc()
    in_maps = []
    for c in range(N_CORES):
        b, g = divmod(c, 4)
        wqk_slice = w_qk[:, 512 * g:512 * (g + 1)].reshape(D, 8, DIM)
        wqk_perm_f32 = wqk_slice[:, perm].reshape(D, 512)
        # augment w_v with per-group column sums of (permuted) w_qk: the v
        # projection then also emits the layernorm group sums
        gsums = wqk_perm_f32.reshape(D, 8, DIM).sum(axis=2)
        wvx = np.concatenate(
            [w_v[:, 256 * g:256 * (g + 1)], gsums.astype(bf)], axis=1)
        in_maps.append({
            "xt": xT[b],
            "w_qk": np.ascontiguousarray(wqk_perm_f32).astype(bf),
            "w_v": np.ascontiguousarray(wvx),
            "w_out": np.ascontiguousarray(w_out[256 * g:256 * (g + 1), :]),
        })

    from concourse.bass_utils import run_bass_kernel_spmd

    trace = bool(os.environ.get("KERNEL_TRACE"))
    if trace:
        try:
            from antenv.axon_hooks import get_axon_ntff_profile_hook  # noqa: F401
        except Exception:
            trace = False
    res = run_bass_kernel_spmd(nc, in_maps, core_ids=list(range(N_CORES)),
                               trace=trace)
    _CACHE["last_results"] = res
    outs = [np.asarray(m["out"], dtype=np.float32) for m in res.results]
    out = np.stack([
        outs[0] + outs[1] + outs[2] + outs[3],
        outs[4] + outs[5] + outs[6] + outs[7],
    ]).astype(np.float32)
    return out


# revision 20
# speedup vs baseline: 2.0172x; 1.0598x over previous
"""Trainium2 Bass kernel for nn_Causal_Attention_13082470383895.

Full (unsharded) inputs in, full output out. Internally shards batch*heads
across 8 NeuronCores: core c owns batch c//4 and the 4 heads [4*(c%4), 4*(c%4)+4).
Each core computes its heads' q/k/v projections, QK-layernorm, causal
unnormalized-exp attention, and its partial contribution to the output
projection (row-sharded W_out). Host sums the 4 partials per batch.

All matmul operands are bf16 (PE runs 1 cycle/row vs 2-4 for fp32 modes).
Host-side prep: x is pre-transposed to [D, L] (removes 128 on-chip PE
transposes), weights are pre-scaled by 1/32 so no device-side scaling is
needed, and W_qk columns are permuted to [q0,q1,k0,k1,q2,q3,k2,k3] so each
128-wide transpose slab directly yields a head-paired q^T/k^T tile.

Hardcoded shapes (per spec): inputs [2, 2048, 1024], W_qk [1024, 2048],
W_v [1024, 1024], W_out [1024, 1024], q/k scale=ones, bias=zeros (per spec
fill; layernorm affine is identity and is not applied).
"""

import os
import sys

import numpy as np

sys.path.insert(0, "/opt/trn_rl_repo")

B = 2
L = 2048
D = 1024
HEADS = 16
DIM = 64
LN_EPS = 1e-6
P = 128
LT = L // P          # 16 l-tiles
DT = D // P          # 8 contraction tiles
NHL = 4              # heads per core
SUP = 4              # 512-wide l_q supertiles
N_CORES = 8

_CACHE = {}


def _make_bacc_cls():
    import bass_rust
    import concourse.mybir as mybir
    from concourse import bacc
    from concourse.hw_specs import get_activation_tables

    class KernelBacc(bacc.Bacc):
        """Bacc whose ACT-table selector never picks the `natural_log` set
        for Ln: hiding `ln` there makes the greedy selector choose
        `natural_log_exp_and_others` (which also holds exp/copy), so the
        kernel needs a single table load instead of thrashing
        exp_and_others <-> natural_log on every layernorm."""

        def insert_act_table_loads(self):
            has_activation = any(
                isinstance(i, mybir.InstActivation)
                for b in self.main_func.blocks
                for i in b.instructions
            )
            if not has_activation:
                return
            ln = mybir.ActivationFunctionType.Ln
            tables = []
            for name, funcs in get_activation_tables(self.m.arch).items():
                if name == "natural_log":
                    funcs = funcs - {ln}
                tables.append((name, funcs))
            bass_rust.insert_act_table_loads(self, tables)

    return KernelBacc


def _build_nc():
    import concourse.bass as bass
    import concourse.mybir as mybir
    import concourse.tile as tile
    from concourse.masks import make_identity, make_upper_triangular

    f32 = mybir.dt.float32
    f32r = mybir.dt.float32r
    bf16 = mybir.dt.bfloat16
    AF = mybir.ActivationFunctionType
    ALU = mybir.AluOpType

    nc = _make_bacc_cls()("TRN2", target_bir_lowering=False, debug=False)

    debug = bool(os.environ.get("KERNEL_DEBUG"))
    XT = nc.dram_tensor("xt", [D, L], bf16, kind="ExternalInput").ap()
    WQK = nc.dram_tensor("w_qk", [D, 512], bf16, kind="ExternalInput").ap()
    # w_v is augmented with 8 columns of per-group column-sums of w_qk, so
    # the v projection also yields the layernorm group sums for free.
    WV = nc.dram_tensor("w_v", [D, 264], bf16, kind="ExternalInput").ap()
    WOUT = nc.dram_tensor("w_out", [256, D], bf16, kind="ExternalInput").ap()
    OUT = nc.dram_tensor("out", [L, D], bf16, kind="ExternalOutput").ap()
    if debug:
        DBG_QT = nc.dram_tensor("dbg_qt", [P, L], bf16,
                                kind="ExternalOutput").ap()
        DBG_KT = nc.dram_tensor("dbg_kt", [P, L], bf16,
                                kind="ExternalOutput").ap()
        DBG_AT = nc.dram_tensor("dbg_at", [P, L], bf16,
                                kind="ExternalOutput").ap()
        DBG_V = nc.dram_tensor("dbg_v", [P, LT * NHL * (DIM + 1)], bf16,
                               kind="ExternalOutput").ap()

    with tile.TileContext(nc) as tc:
        const = tc.alloc_tile_pool(name="const", bufs=1)
        big = tc.alloc_tile_pool(name="big", bufs=1)
        qkp = tc.alloc_tile_pool(name="qkp", bufs=5)
        stat = tc.alloc_tile_pool(name="stat", bufs=3)
        esp = tc.alloc_tile_pool(name="esp", bufs=3)
        tailp = tc.alloc_tile_pool(name="tailp", bufs=2)
        outp = tc.alloc_tile_pool(name="outp", bufs=4)

        ident = const.tile([P, P], bf16)
        make_identity(nc, ident)
        # 0/1 upper-triangular (incl diag) mask for post-exp causal masking
        # of diagonal score tiles; layout (k_partition, q_free), valid q>=k.
        up01 = const.tile([P, P], bf16)
        make_upper_triangular(nc, up01, val=1.0, diag=True)
        ones_f32 = const.tile([P, 1], f32)
        nc.vector.memset(ones_f32, 1.0)
        ones_row = const.tile([1, DIM], f32r)
        nc.vector.tensor_copy(ones_row, ones_f32[0:1, :].to_broadcast([1, DIM]))
        epsb = const.tile([P, 1], f32)
        nc.vector.memset(epsb, float(LN_EPS))

        tc.strict_bb_all_engine_barrier()

        # --- weights + x^T DMA (bf16 straight from host, no casts) -------
        # fine-grained pieces across the 16 DMA queues, ordered so the first
        # supertile's v/qk projections can start after ~one queue round
        wqk = big.tile([P, DT, 512], bf16)
        wv = big.tile([P, DT, 264], bf16)
        xt_all = big.tile([P, DT, L], bf16)
        for d in range(DT):
            nc.sync.dma_start(wv[:, d], WV[d * P:(d + 1) * P, :])
            nc.sync.dma_start(wqk[:, d], WQK[d * P:(d + 1) * P, :])
            nc.sync.dma_start(xt_all[:, d, :512], XT[d * P:(d + 1) * P, :512])
        wout = big.tile([P, 2, D], bf16)
        for c in range(2):
            nc.sync.dma_start(wout[:, c], WOUT[c * P:(c + 1) * P, :])
        for q in range(1, 4):
            for d in range(DT):
                nc.sync.dma_start(xt_all[:, d, q * 512:(q + 1) * 512],
                                  XT[d * P:(d + 1) * P, q * 512:(q + 1) * 512])

        # persistent intermediates. qt/kt/at pair 2 heads on the partition
        # axis: local head 2i in rows 0:64, head 2i+1 in rows 64:128.
        # v is stored augmented per head: [v_h | 1] (65 cols) so one AV
        # matmul yields both the numerator (rows 0:64) and the softmax
        # denominator (row 64).
        v_sb = big.tile([P, LT, NHL, DIM + 1], bf16)
        nc.vector.memset(v_sb[:, :, :, DIM], 1.0)
        qt = [big.tile([P, L], bf16, name=f"qt{i}") for i in range(2)]
        kt = [big.tile([P, L], bf16, name=f"kt{i}") for i in range(2)]
        at = [big.tile([P, L], bf16, name=f"at{i}") for i in range(2)]

        # PSUM: tag "big" [P,2,512]f32 = 2 banks x bufs2 = 4 (st tiles in
        # the attention j-loop, op tiles in the out-projection); tag "av"
        # [65,512]f32 1 bank x bufs2 = 2; tag "work" [P,512]f32 1 bank x
        # bufs2 = 2 (qk/v projections and q/k transposes). Total 8 banks.
        with tc.tile_pool(name="ps", bufs=2, space="PSUM") as ps:

            def phase_a(s):
                qk_tiles = []
                for t in range(4 * s, 4 * s + 4):
                    # v projection first: its last 8 cols are the layernorm
                    # group sums of the (not yet computed) qk projection
                    v_ps = ps.tile([P, 512], f32, tag="work", name="v_ps")
                    for d in range(DT):
                        nc.tensor.matmul(
                            v_ps[:, :264], xt_all[:, d, t * P:(t + 1) * P],
                            wv[:, d],
                            start=(d == 0), stop=(d == DT - 1),
                        )
                    nc.vector.tensor_copy(
                        v_sb[:, t, :, :DIM],
                        v_ps[:, :256].rearrange("p (h d) -> p h d", h=NHL))
                    sums = v_ps[:, 256:264]
                    mean = stat.tile([P, 8, 1], f32, tag="mean")
                    nc.vector.tensor_scalar(mean[:, :, 0], sums,
                                            1.0 / DIM, None, op0=ALU.mult)

                    qk_ps = ps.tile([P, 512], f32, tag="work", name="qk_ps")
                    for d in range(DT):
                        nc.tensor.matmul(
                            qk_ps, xt_all[:, d, t * P:(t + 1) * P],
                            wqk[:, d],
                            start=(d == 0), stop=(d == DT - 1),
                        )
                    # grouped layernorm: var = (sumsq - sums*mean)/64 + eps,
                    # the /64 folded into the Ln scale. Apply is two
                    # full-width ops with [P,8,1]->[P,8,64] broadcasts.
                    qk_g = qk_ps.rearrange("p (g d) -> p g d", g=8)
                    sq = qkp.tile([P, 512], f32, tag="sq", bufs=2, name="sq")
                    nc.scalar.activation(sq, qk_ps, AF.Square, scale=1.0)
                    sumsq = stat.tile([P, 8], f32, tag="sumsq")
                    nc.vector.tensor_reduce(
                        sumsq, sq.rearrange("p (g d) -> p g d", g=8),
                        mybir.AxisListType.X, ALU.add)
                    u = stat.tile([P, 8], f32, tag="u")
                    nc.vector.tensor_tensor(u, sums, mean[:, :, 0], ALU.mult)
                    nc.vector.tensor_tensor(u, sumsq, u, ALU.subtract)
                    rstd = stat.tile([P, 8, 1], f32, tag="rstd")
                    nc.scalar.activation(rstd[:, :, 0], u, AF.Ln,
                                         bias=epsb, scale=1.0 / DIM)
                    nc.scalar.activation(rstd[:, :, 0], rstd[:, :, 0],
                                         AF.Exp, scale=-0.5)
                    qk_sb = qkp.tile([P, 512], bf16, tag="qk", name="qk_sb")
                    qk_sbg = qk_sb.rearrange("p (g d) -> p g d", g=8)
                    cen = qkp.tile([P, 512], f32, tag="cen", bufs=2,
                                   name="cen")
                    ceng = cen.rearrange("p (g d) -> p g d", g=8)
                    nc.vector.tensor_tensor(
                        ceng, qk_g, mean.to_broadcast([P, 8, DIM]),
                        ALU.subtract)
                    nc.vector.tensor_tensor(
                        qk_sbg, ceng, rstd.to_broadcast([P, 8, DIM]),
                        ALU.mult)
                    qk_tiles.append(qk_sb)

                # q/k transposes. Host permuted W_qk columns so slab a of
                # qk_sb is [q_even|q_odd] or [k_even|k_odd] of a head pair:
                # one [128,128] transpose yields the paired-layout tile.
                for a, dst in ((0, qt[0]), (1, kt[0]), (2, qt[1]), (3, kt[1])):
                    tp = ps.tile([P, 512], bf16, tag="work", name="tp_ps")
                    for i in range(4):
                        nc.tensor.transpose(
                            tp[:, i * P:(i + 1) * P],
                            qk_tiles[i][:, a * P:(a + 1) * P],
                            ident,
                        )
                    nc.vector.tensor_copy(dst[:, s * 512:(s + 1) * 512], tp)

            def tail_late(s, pr, dens, avsbs):
                # softmax tail: recip + broadcast + apply. Emitted under PE
                # cover of the following pair's j-loop / phase_c.
                for r01 in range(2):
                    ro = DIM * r01
                    rec = stat.tile([1, 512], f32, tag="rec")
                    nc.vector.reciprocal_approx_fast(rec, dens[r01])
                    rec_r = stat.tile([1, 512], f32r, tag="recr")
                    nc.gpsimd.tensor_copy(rec_r, rec)
                    bc = ps.tile([DIM, 512], f32, tag="big", name="bc")
                    nc.tensor.matmul(bc, ones_row, rec_r,
                                     start=True, stop=True)
                    nc.vector.tensor_tensor(
                        at[pr][ro:ro + DIM, s * 512:(s + 1) * 512],
                        avsbs[r01], bc, ALU.mult)

            def phase_bc(s, pending):
                njs = 4 * s + 4
                for pr in range(2):
                    av = [ps.tile([DIM + 1, 512], f32, tag="av",
                                  name=f"av{r01}") for r01 in range(2)]
                    for j in range(njs):
                        pp = j - 4 * s
                        woff = max(0, pp) * P
                        st = ps.tile([P, 2, 512], f32, tag="big", name="st")
                        for r01 in range(2):
                            ro = DIM * r01
                            nc.tensor.matmul(
                                st[:, r01, woff:],
                                kt[pr][ro:ro + DIM, j * P:(j + 1) * P],
                                qt[pr][ro:ro + DIM,
                                       s * 512 + woff:(s + 1) * 512],
                                start=True, stop=True, tile_position=(ro, 0),
                            )
                        es = esp.tile([P, 2, 512], bf16, tag="es")
                        nc.scalar.activation(es[:, :, woff:], st[:, :, woff:],
                                             AF.Exp, scale=1.0 / DIM)
                        if pp >= 0:
                            blk = slice(pp * P, (pp + 1) * P)
                            nc.gpsimd.tensor_tensor(
                                es[:, :, blk], es[:, :, blk],
                                up01.unsqueeze(1).to_broadcast([P, 2, P]),
                                ALU.mult)
                        for r01 in range(2):
                            nc.tensor.matmul(
                                av[r01][:, woff:],
                                v_sb[:, j, 2 * pr + r01],
                                es[:, r01, woff:],
                                start=(j == 0), stop=(j == njs - 1),
                            )
                        if j == 1 and pending is not None:
                            pending()
                            pending = None
                    # stage the denominator row (custom-DVE recip mis-reads
                    # PSUM at a nonzero base partition on HW) and the av
                    # numerator, releasing the av PSUM banks for the next
                    # pair; the recip/bc/at chain is deferred.
                    dens, avsbs = [], []
                    for r01 in range(2):
                        den = stat.tile([1, 512], f32, tag="den")
                        nc.scalar.copy(den, av[r01][DIM:DIM + 1, :])
                        av_sb = tailp.tile([DIM, 512], f32, tag="avsb")
                        nc.vector.tensor_copy(av_sb, av[r01][:DIM])
                        dens.append(den)
                        avsbs.append(av_sb)
                    pending = (lambda s=s, pr=pr, dens=dens, avsbs=avsbs:
                               tail_late(s, pr, dens, avsbs))
                return pending

            def phase_c(s, pending=None):
                # pending writes this supertile's at columns when s==SUP-1,
                # so it must land before the op matmuls that read them
                if pending is not None:
                    pending()
                for t in range(4 * s, 4 * s + 4):
                    op = ps.tile([P, 2, 512], f32, tag="big", name="op")
                    for nch in range(2):
                        for c in range(2):
                            nc.tensor.matmul(
                                op[:, nch],
                                at[c][:, t * P:(t + 1) * P],
                                wout[:, c, nch * 512:(nch + 1) * 512],
                                start=(c == 0), stop=(c == 1),
                            )
                    o_sb = outp.tile([P, D], bf16, tag="o")
                    nc.scalar.copy(o_sb, op.rearrange("p a b -> p (a b)"))
                    nc.sync.dma_start(OUT[t * P:(t + 1) * P, :512],
                                      o_sb[:, :512])
                    nc.sync.dma_start(OUT[t * P:(t + 1) * P, 512:],
                                      o_sb[:, 512:])

            pending = None
            for s in range(SUP):
                phase_a(s)
                pending = phase_bc(s, pending)
                if s > 0:
                    phase_c(s - 1)
            phase_c(SUP - 1, pending)
            if debug:
                nc.sync.dma_start(DBG_QT, qt[0])
                nc.sync.dma_start(DBG_KT, kt[0])
                nc.sync.dma_start(DBG_AT, at[0])
                nc.sync.dma_start(
                    DBG_V, v_sb.rearrange("p a b c -> p (a b c)"))

        outp.release()
        tailp.release()
        esp.release()
        stat.release()
        qkp.release()
        big.release()
        const.release()

    nc.finalize()
    return nc


def _get_nc():
    if "nc" not in _CACHE:
        _CACHE["nc"] = _build_nc()
    return _CACHE["nc"]


def kernel(**inputs):
    import ml_dtypes

    bf = ml_dtypes.bfloat16
    x = np.asarray(inputs["inputs"], dtype=np.float32)
    w_qk = np.asarray(inputs["W_qk"], dtype=np.float32) * (1.0 / 32.0)
    w_v = (np.asarray(inputs["W_v"], dtype=np.float32) * (1.0 / 32.0)).astype(bf)
    w_out = (np.asarray(inputs["W_out"], dtype=np.float32) * (1.0 / 32.0)).astype(bf)

    # permute each core's 8 qk column-groups [q0,k0,q1,k1,q2,k2,q3,k3] ->
    # [q0,q1,k0,k1,q2,q3,k2,k3] so a 128-col transpose slab is a head pair
    perm = [0, 2, 1, 3, 4, 6, 5, 7]
    xT = [np.ascontiguousarray(x[b].T).astype(bf) for b in range(B)]

    nc = _get_nc()
    in_maps = []
    for c in range(N_CORES):
        b, g = divmod(c, 4)
        wqk_slice = w_qk[:, 512 * g:512 * (g + 1)].reshape(D, 8, DIM)
        wqk_perm_f32 = wqk_slice[:, perm].reshape(D, 512)
        # augment w_v with per-group column sums of (permuted) w_qk: the v
        # projection then also emits the layernorm group sums
        gsums = wqk_perm_f32.reshape(D, 8, DIM).sum(axis=2)
        wvx = np.concatenate(
            [w_v[:, 256 * g:256 * (g + 1)], gsums.astype(bf)], axis=1)
        in_maps.append({
            "xt": xT[b],
            "w_qk": np.ascontiguousarray(wqk_perm_f32).astype(bf),
            "w_v": np.ascontiguousarray(wvx),
            "w_out": np.ascontiguousarray(w_out[256 * g:256 * (g + 1), :]),
        })

    from concourse.bass_utils import run_bass_kernel_spmd

    trace = bool(os.environ.get("KERNEL_TRACE"))
    if trace:
        try:
            from antenv.axon_hooks import get_axon_ntff_profile_hook  # noqa: F401
        except Exception:
            trace = False
    res = run_bass_kernel_spmd(nc, in_maps, core_ids=list(range(N_CORES)),
                               trace=trace)
    _CACHE["last_results"] = res
    outs = [np.asarray(m["out"], dtype=np.float32) for m in res.results]
    out = np.stack([
        outs[0] + outs[1] + outs[2] + outs[3],
        outs[4] + outs[5] + outs[6] + outs[7],
    ]).astype(np.float32)
    return out
